# revision 34
# baseline (speedup 1.0000x reference)
"""Trainium2 Bass kernel for the 2-layer contractive autoencoder (CAE).

reference math (B=512, D=1024, H1=512, H2=128):
    c1  = sigmoid(x @ W1.T + b1)          [B, H1]
    c2  = sigmoid(c1 @ W2.T + b2)         [B, H2]
    c3  = sigmoid(c2 @ W2 + b3)           [B, H1]
    rec = c3 @ W1 + b_r                   [B, D]
    Jac[b] = diag(s2p[b]) @ W2 @ diag(s1p[b]) @ W1     [B, H2, D]
      with s1p = c1*(1-c1), s2p = c2*(1-c2)

Sharding: data-parallel over the batch dim across 8 NeuronCores (64 rows
per core); W1/W2/biases replicated. Everything on-chip per core.

Per-core layout strategy (partition dim first):
  - encoder computed transposed (hidden dim on partitions) so that
    s1p/s2p land as per-partition scalar columns:
        c1T [H1, BL], c2T [H2, BL], c3T [H1, BL]
  - Jac[b] = M_b @ W1 with M_bT = W2T * s1p[b] (per-partition scale,
    DVE, cast to fp16) as the matmul stationary operand and W1 (natural
    layout, fp16) as the moving operand; the s2p[b] scale is fused into
    the mandatory PSUM->SBUF copy (ACT, per-partition scale).
  - derivative tensors are stored negated ((c-1)*c = -c(1-c), one DVE
    op each); the two negations cancel in Jac.
  - encoder matmuls + transposes in fp32 for accuracy (tiny FLOP count);
    the 8.6 GFLOP/core Jacobian einsum and the recover matmul run in
    fp16 (same 1 cycle/row PE rate as bf16, 8x finer mantissa; all
    values are far inside fp16 range).
"""

from contextlib import ExitStack

import numpy as np

import concourse.bass as bass
import concourse.tile as tile
from concourse import bacc
from concourse import masks, mybir
from concourse.bass_utils import run_bass_kernel_spmd

F32 = mybir.dt.float32
F16 = mybir.dt.float16  # same PE rate as bf16, 8x finer mantissa
AF = mybir.ActivationFunctionType
ALU = mybir.AluOpType

B, D, H1, H2 = 512, 1024, 512, 128
NCORES = 8
BL = B // NCORES  # 64 batch rows per core
P = 128           # partitions
K1 = H1 // P      # 4 k-tiles over H1
KD = D // P       # 8 k-tiles over D
NB = 512          # fp32 psum bank width (free dim)
ND = D // NB      # 2 n-tiles over D
JDMA = 2          # jac batches per output DMA (1MiB each)


def _build() -> bass.Bass:
    nc = bacc.Bacc("TRN2")

    x_d = nc.dram_tensor("x", [BL, D], F32, kind="ExternalInput")
    w1_d = nc.dram_tensor("w1", [H1, D], F32, kind="ExternalInput")
    b1_d = nc.dram_tensor("b1", [H1], F32, kind="ExternalInput")
    w2_d = nc.dram_tensor("w2", [H2, H1], F32, kind="ExternalInput")
    b2_d = nc.dram_tensor("b2", [H2], F32, kind="ExternalInput")
    b3_d = nc.dram_tensor("b3", [H1], F32, kind="ExternalInput")
    br_d = nc.dram_tensor("br", [D], F32, kind="ExternalInput")

    rec_d = nc.dram_tensor("rec", [BL, D], F32, kind="ExternalOutput")
    c2_d = nc.dram_tensor("c2o", [BL, H2], F32, kind="ExternalOutput")
    jac_d = nc.dram_tensor("jac", [BL, H2, D], F32, kind="ExternalOutput")

    with tile.TileContext(nc) as tc, ExitStack() as ctx:
        const = ctx.enter_context(tc.tile_pool(name="const", bufs=1))
        work = ctx.enter_context(tc.tile_pool(name="work", bufs=3))
        lhsp = ctx.enter_context(tc.tile_pool(name="lhsp", bufs=4))
        jop = ctx.enter_context(tc.tile_pool(name="jop", bufs=3))
        pp = ctx.enter_context(tc.tile_pool(name="pp", bufs=4, space="PSUM"))
        jp = ctx.enter_context(tc.tile_pool(name="jp", bufs=2, space="PSUM"))

        ident = const.tile([P, P], F32, name="ident", tag="ident")
        masks.make_identity(nc, ident[:])

        # ---- PE warm-up: ~3us of cheap fp16 dummy matmuls so the PE clock
        # ramp (HAM) is fully open by the time the first input DMA lands;
        # results are discarded ----
        warm = const.tile([P, NB], F16, name="warm", tag="warm")
        nc.gpsimd.memset(warm[:], 0.0)
        for _ in range(4):
            wps = pp.tile([P, NB], F32, name="wps", tag="pp")
            nc.tensor.matmul(wps[:], lhsT=warm[:, :P], rhs=warm[:],
                             start=True, stop=True)

        # ---- load x first (small, one DMA) so PE can start transposing ----
        xin = work.tile([BL, D], F32, name="xin", tag="xin", bufs=1)
        nc.sync.dma_start(xin[:], x_d[:, :])

        # ---- load weights / biases ----
        w1f = []
        for k in range(K1):
            t = const.tile([P, D], F32, name=f"w1f{k}", tag=f"w1f{k}")
            nc.sync.dma_start(t[:], w1_d[k * P:(k + 1) * P, :])
            w1f.append(t)
        w2f = const.tile([P, H1], F32, name="w2f", tag="w2f")
        nc.sync.dma_start(w2f[:], w2_d[:, :])

        b1t = const.tile([P, K1], F32, name="b1t", tag="b1t")
        nc.sync.dma_start(b1t[:], b1_d[:].rearrange("(k p) -> p k", p=P))
        b3t = const.tile([P, K1], F32, name="b3t", tag="b3t")
        nc.sync.dma_start(b3t[:], b3_d[:].rearrange("(k p) -> p k", p=P))
        b2t = const.tile([P, 1], F32, name="b2t", tag="b2t")
        nc.sync.dma_start(b2t[:], b2_d[:].rearrange("(p o) -> p o", o=1))
        brf = const.tile([1, D], F32, name="brf", tag="brf")
        nc.sync.dma_start(brf[:], br_d[:].rearrange("(o d) -> o d", o=1))
        brt = const.tile([1, D], F16, name="brt", tag="brt")
        nc.vector.tensor_copy(brt[:], brf[:])
        ones_t = const.tile([1, BL], F16, name="ones", tag="ones")
        nc.vector.memset(ones_t[:], 1.0)

        # ---- W1 cast to fp16 (moving operand of the Jacobian matmuls) ----
        w1b = []
        for k in range(K1):
            t = const.tile([P, D], F16, name=f"w1b{k}", tag=f"w1b{k}")
            nc.scalar.copy(t[:], w1f[k][:])
            w1b.append(t)

        # ---- transposes (PE identity-matmul, fp32) ----
        # x [BL, D] -> xT k-tiles [P, BL]
        xT = []
        for k in range(KD):
            ps = pp.tile([P, BL], F32, name="pp", tag="pp")
            nc.tensor.transpose(ps[:], xin[:, k * P:(k + 1) * P], ident[:BL, :BL])
            t = const.tile([P, BL], F32, name=f"xT{k}", tag=f"xT{k}")
            nc.vector.tensor_copy(t[:], ps[:])
            xT.append(t)

        # W1 [H1, D] -> W1T k-tiles [P(D), H1]
        w1T = [const.tile([P, H1], F32, name=f"w1T{k}", tag=f"w1T{k}") for k in range(KD)]
        for m in range(K1):
            for k in range(KD):
                ps = pp.tile([P, P], F32, name="pp", tag="pp")
                nc.tensor.transpose(ps[:], w1f[m][:, k * P:(k + 1) * P], ident[:])
                nc.vector.tensor_copy(w1T[k][:, m * P:(m + 1) * P], ps[:])

        # W2 [H2, H1] -> W2T k-tiles [P(H1), H2]
        w2T = []
        for k in range(K1):
            ps = pp.tile([P, P], F32, name="pp", tag="pp")
            nc.tensor.transpose(ps[:], w2f[:, k * P:(k + 1) * P], ident[:])
            t = const.tile([P, H2], F32, name=f"w2T{k}", tag=f"w2T{k}")
            nc.vector.tensor_copy(t[:], ps[:])
            w2T.append(t)

        # ---- encoder (fp32) ----
        # c1T[m] = sigmoid(W1 @ xT + b1)   [P, BL] per H1-tile m
        c1T = [const.tile([P, BL], F32, name=f"c1T{m}", tag=f"c1T{m}") for m in range(K1)]
        s1n = [const.tile([P, BL], F32, name=f"s1n{m}", tag=f"s1n{m}") for m in range(K1)]  # -s1p
        for m in range(K1):
            ps = pp.tile([P, BL], F32, name="pp", tag="pp")
            for k in range(KD):
                nc.tensor.matmul(
                    ps[:], lhsT=w1T[k][:, m * P:(m + 1) * P], rhs=xT[k][:],
                    start=(k == 0), stop=(k == KD - 1),
                )
            nc.scalar.activation(c1T[m][:], ps[:], AF.Sigmoid, bias=b1t[:, m:m + 1])
            # -s1p = (c1 - 1) * c1
            nc.vector.scalar_tensor_tensor(
                s1n[m][:], c1T[m][:], 1.0, c1T[m][:], ALU.subtract, ALU.mult,
            )

        # c2T = sigmoid(W2 @ c1T... ) : lhsT = W2T tiles, rhs = c1T tiles
        c2Tt = const.tile([P, BL], F32, name="c2T", tag="c2T")
        s2n = const.tile([P, BL], F32, name="s2n", tag="s2n")  # -s2p
        ps = pp.tile([P, BL], F32, name="pp", tag="pp")
        for k in range(K1):
            nc.tensor.matmul(
                ps[:], lhsT=w2T[k][:], rhs=c1T[k][:],
                start=(k == 0), stop=(k == K1 - 1),
            )
        nc.scalar.activation(c2Tt[:], ps[:], AF.Sigmoid, bias=b2t[:, 0:1])
        nc.vector.scalar_tensor_tensor(
            s2n[:], c2Tt[:], 1.0, c2Tt[:], ALU.subtract, ALU.mult,
        )

        def emit_decoder():
            # c2 natural output [BL, H2]
            psc = pp.tile([BL, H2], F32, name="psc", tag="pp")
            nc.tensor.transpose(psc[:], c2Tt[:], ident[:])
            c2n = work.tile([BL, H2], F32, name="c2n", tag="c2n")
            nc.vector.tensor_copy(c2n[:], psc[:])
            nc.sync.dma_start(c2_d[:, :], c2n[:])

            # c3T[m] = sigmoid(W2.T @ c2T + b3) : lhsT = W2 natural slices
            # (fp16 output: feeds the fp16 recover matmul)
            c3T = [const.tile([P, BL], F16, name=f"c3T{m}", tag=f"c3T{m}")
                   for m in range(K1)]
            for m in range(K1):
                ps = pp.tile([P, BL], F32, name="psd", tag="pp")
                nc.tensor.matmul(
                    ps[:], lhsT=w2f[:, m * P:(m + 1) * P], rhs=c2Tt[:],
                    start=True, stop=True,
                )
                nc.scalar.activation(c3T[m][:], ps[:], AF.Sigmoid, bias=b3t[:, m:m + 1])

            # recover = c3 @ W1 + b_r  (natural [BL, D], fp16 matmul;
            # b_r added via a K=1 rank-1 matmul into the same PSUM group)
            rec_sb = work.tile([BL, D], F32, name="rec", tag="rec")
            for n in range(ND):
                ps = pp.tile([BL, NB], F32, name="psr", tag="pp")
                for k in range(K1):
                    nc.tensor.matmul(
                        ps[:], lhsT=c3T[k][:], rhs=w1b[k][:, n * NB:(n + 1) * NB],
                        start=(k == 0), stop=False,
                    )
                nc.tensor.matmul(
                    ps[:], lhsT=ones_t[:], rhs=brt[:, n * NB:(n + 1) * NB],
                    start=False, stop=True,
                )
                nc.vector.tensor_copy(rec_sb[:, n * NB:(n + 1) * NB], ps[:])
            nc.sync.dma_start(rec_d[:, :], rec_sb[:])

        # ---- Jacobian loop (fp16 matmuls) ----
        # (the decoder/recover emission is deferred into the loop so the Jac
        # output DMA stream starts as early as possible)
        # Jac[b] = (-s2p[b]) * [ ((W2T * -s1p[b]).T ) @ W1 ]
        # k-outer / n-inner: each stationary lhs tile is loaded once and
        # streams both 512-wide halves of W1.
        for b0 in range(0, BL, JDMA):
            jt = jop.tile([P, JDMA * D], F32, name="jo", tag="jo")
            for bi in range(JDMA):
                b = b0 + bi
                lhs = []
                for k in range(K1):
                    lt = lhsp.tile([P, H2], F16, name=f"lhs{k}", tag=f"lhs{k}")
                    nc.vector.tensor_scalar_mul(lt[:], w2T[k][:], s1n[k][:, b:b + 1])
                    lhs.append(lt)
                pss = [jp.tile([P, NB], F32, name=f"jp{n}", tag=f"jp{n}")
                       for n in range(ND)]
                for k in range(K1):
                    for n in range(ND):
                        nc.tensor.matmul(
                            pss[n][:], lhsT=lhs[k][:],
                            rhs=w1b[k][:, n * NB:(n + 1) * NB],
                            start=(k == 0), stop=(k == K1 - 1),
                        )
                last = b == BL - 1
                for n in range(ND):
                    dst = jt[:, bi * D + n * NB: bi * D + (n + 1) * NB]
                    if last and n == 1:
                        # split the final batch's drain across ACT+DVE so the
                        # kernel tail is shorter
                        nc.vector.tensor_scalar_mul(dst, pss[n][:], s2n[:, b:b + 1])
                    else:
                        nc.scalar.mul(dst, pss[n][:], s2n[:, b:b + 1])
            if b0 >= BL - 2 * JDMA:
                for bi in range(JDMA):
                    nc.sync.dma_start(
                        jac_d[b0 + bi],
                        jt[:, bi * D:(bi + 1) * D],
                    )
            else:
                nc.sync.dma_start(
                    jac_d[b0:b0 + JDMA].rearrange("b h d -> h b d"),
                    jt[:].rearrange("p (b d) -> p b d", b=JDMA),
                )
            if b0 == 0:
                emit_decoder()

    nc.compile()
    return nc


_CACHE: dict = {}


def _get_nc() -> bass.Bass:
    if "nc" not in _CACHE:
        _CACHE["nc"] = _build()
    return _CACHE["nc"]


def _in_maps(x, W1, b1, W2, b2, b3, b_r):
    def f(a):
        return np.ascontiguousarray(np.asarray(a, dtype=np.float32))

    x, W1, b1, W2, b2, b3, b_r = map(f, (x, W1, b1, W2, b2, b3, b_r))
    maps = []
    for c in range(NCORES):
        maps.append({
            "x": x[c * BL:(c + 1) * BL],
            "w1": W1, "b1": b1, "w2": W2, "b2": b2, "b3": b3, "br": b_r,
        })
    return maps


def _gather(results):
    recover = np.concatenate([results[c]["rec"] for c in range(NCORES)], axis=0)
    c2 = np.concatenate([results[c]["c2o"] for c in range(NCORES)], axis=0)
    jac = np.concatenate([results[c]["jac"] for c in range(NCORES)], axis=0)
    return recover, c2, jac


def kernel(x, W1, b1, W2, b2, b3, b_r):
    nc = _get_nc()
    maps = _in_maps(x, W1, b1, W2, b2, b3, b_r)
    res = run_bass_kernel_spmd(nc, maps, list(range(NCORES)))
    return _gather(res.results)


def kernel_profiled(x, W1, b1, W2, b2, b3, b_r, **kw):
    """Same as kernel() but with NTFF tracing; returns (outputs, results)."""
    nc = _get_nc()
    maps = _in_maps(x, W1, b1, W2, b2, b3, b_r)
    res = run_bass_kernel_spmd(nc, maps, list(range(NCORES)), trace=True, **kw)
    return _gather(res.results), res


def _make_sharded(nc):
    """Build the same shard_map'ed PJRT callable bass2jax uses, without
    output donation, so it can be re-invoked for timing."""
    import jax
    import numpy as jnp_np
    from jax.sharding import Mesh, PartitionSpec
    from jax.experimental.shard_map import shard_map
    from concourse import bass2jax, mybir as mb

    bass2jax.install_neuronx_cc_hook()
    in_names, out_names, out_avals = [], [], []
    partition_name = nc.partition_id_tensor.name if nc.partition_id_tensor else None
    for alloc in nc.m.functions[0].allocations:
        if not isinstance(alloc, mb.MemoryLocationSet):
            continue
        name = alloc.memorylocations[0].name
        if alloc.kind == "ExternalInput":
            if name != partition_name:
                in_names.append(name)
        elif alloc.kind == "ExternalOutput":
            out_names.append(name)
            out_avals.append(jax.core.ShapedArray(
                tuple(alloc.tensor_shape), mb.dt.np(alloc.dtype)))
    n_params = len(in_names)
    all_in_names = list(in_names) + list(out_names)
    if partition_name is not None:
        all_in_names.append(partition_name)

    def _body(*args):
        operands = list(args)
        if partition_name is not None:
            operands.append(bass2jax.partition_id_tensor())
        return tuple(bass2jax._bass_exec_p.bind(
            *operands,
            out_avals=tuple(out_avals),
            in_names=tuple(all_in_names),
            out_names=tuple(out_names),
            lowering_input_output_aliases=(),
            sim_require_finite=True,
            sim_require_nnan=True,
            nc=nc,
        ))

    devices = jax.devices()[:NCORES]
    mesh = Mesh(jnp_np.asarray(devices), ("core",))
    n_outs = len(out_names)
    in_specs = (PartitionSpec("core"),) * (n_params + n_outs)
    out_specs = (PartitionSpec("core"),) * n_outs
    fn = jax.jit(shard_map(_body, mesh=mesh, in_specs=in_specs,
                           out_specs=out_specs, check_rep=False),
                 keep_unused=True)
    return fn, in_names, out_names, out_avals


def measure_chain_ns(x, W1, b1, W2, b2, b3, b_r, chain=8, iters=8):
    """Estimate the marginal on-device execution time of one kernel run by
    timing a jitted program that chains `chain` data-dependent kernel
    executions, vs one with a single execution. The axon-tunnel dispatch
    overhead (~100ms) cancels in the difference."""
    import time as _time

    import jax
    import jax.numpy as jnp
    import numpy as jnp_np
    from jax.sharding import Mesh, PartitionSpec
    from jax.experimental.shard_map import shard_map
    from concourse import bass2jax, mybir as mb

    nc = _get_nc()
    maps = _in_maps(x, W1, b1, W2, b2, b3, b_r)
    bass2jax.install_neuronx_cc_hook()

    in_names, out_names, out_avals = [], [], []
    partition_name = nc.partition_id_tensor.name if nc.partition_id_tensor else None
    for alloc in nc.m.functions[0].allocations:
        if not isinstance(alloc, mb.MemoryLocationSet):
            continue
        name = alloc.memorylocations[0].name
        if alloc.kind == "ExternalInput":
            if name != partition_name:
                in_names.append(name)
        elif alloc.kind == "ExternalOutput":
            out_names.append(name)
            out_avals.append(jax.core.ShapedArray(
                tuple(alloc.tensor_shape), mb.dt.np(alloc.dtype)))
    all_in_names = list(in_names) + list(out_names)
    if partition_name is not None:
        all_in_names.append(partition_name)
    n_params = len(in_names)
    i_x = in_names.index("x")
    i_rec = out_names.index("rec")

    def _one(ins, zeros):
        operands = list(ins) + list(zeros)
        if partition_name is not None:
            operands.append(bass2jax.partition_id_tensor())
        return bass2jax._bass_exec_p.bind(
            *operands,
            out_avals=tuple(out_avals),
            in_names=tuple(all_in_names),
            out_names=tuple(out_names),
            lowering_input_output_aliases=(),
            sim_require_finite=True,
            sim_require_nnan=True,
            nc=nc,
        )

    def _make(n):
        def _body(*args):
            ins = list(args[:n_params])
            zeros = list(args[n_params:])
            outs = _one(ins, zeros)
            for _ in range(n - 1):
                # pure data dependency: feed rec back as x (same shape/dtype)
                ins = list(ins)
                ins[i_x] = outs[i_rec]
                outs = _one(ins, zeros)
            return tuple(outs)
        devices = jax.devices()[:NCORES]
        mesh = Mesh(jnp_np.asarray(devices), ("core",))
        n_outs = len(out_names)
        return jax.jit(shard_map(
            _body, mesh=mesh,
            in_specs=(PartitionSpec("core"),) * (n_params + n_outs),
            out_specs=(PartitionSpec("core"),) * n_outs, check_rep=False),
            keep_unused=True)

    concat_in = [
        jnp_np.concatenate([jnp_np.asarray(maps[c][n]) for c in range(NCORES)], axis=0)
        for n in in_names
    ] + [
        jnp_np.zeros((NCORES * a.shape[0], *a.shape[1:]), a.dtype) for a in out_avals
    ]
    dev_in = [jax.device_put(a) for a in concat_in]
    jax.block_until_ready(dev_in)

    def time_fn(fn):
        jax.block_until_ready(fn(*dev_in))  # compile+warm
        ts = []
        for _ in range(iters):
            t0 = _time.perf_counter()
            jax.block_until_ready(fn(*dev_in))
            ts.append((_time.perf_counter() - t0) * 1e9)
        ts.sort()
        return ts

    t1 = time_fn(_make(1))
    tn = time_fn(_make(chain))
    per = (tn[0] - t1[0]) / (chain - 1)
    per_med = (tn[len(tn) // 2] - t1[len(t1) // 2]) / (chain - 1)
    return per, per_med, t1, tn


def measure_exec_ns(x, W1, b1, W2, b2, b3, b_r, iters=20):
    """Warm wall-clock of the sharded PJRT executable with device-resident
    inputs. Returns (min_ns, median_ns, all_ns)."""
    import time as _time

    import jax
    import numpy as jnp_np

    nc = _get_nc()
    maps = _in_maps(x, W1, b1, W2, b2, b3, b_r)
    fn, in_names, out_names, out_avals = _make_sharded(nc)
    concat_in = [
        jnp_np.concatenate([jnp_np.asarray(maps[c][n]) for c in range(NCORES)], axis=0)
        for n in in_names
    ]
    concat_zeros = [
        jnp_np.zeros((NCORES * a.shape[0], *a.shape[1:]), a.dtype) for a in out_avals
    ]
    dev_in = [jax.device_put(a) for a in concat_in + concat_zeros]
    jax.block_until_ready(dev_in)

    outs = fn(*dev_in)   # compile + warm
    jax.block_until_ready(outs)

    times = []
    for _ in range(iters):
        t0 = _time.perf_counter()
        outs = fn(*dev_in)
        jax.block_until_ready(outs)
        times.append((_time.perf_counter() - t0) * 1e9)
    times.sort()
    return times[0], times[len(times) // 2], times, outs, out_names


# revision 41
# speedup vs baseline: 1.0254x; 1.0254x over previous
"""Trainium2 Bass kernel for the 2-layer contractive autoencoder (CAE).

reference math (B=512, D=1024, H1=512, H2=128):
    c1  = sigmoid(x @ W1.T + b1)          [B, H1]
    c2  = sigmoid(c1 @ W2.T + b2)         [B, H2]
    c3  = sigmoid(c2 @ W2 + b3)           [B, H1]
    rec = c3 @ W1 + b_r                   [B, D]
    Jac[b] = diag(s2p[b]) @ W2 @ diag(s1p[b]) @ W1     [B, H2, D]
      with s1p = c1*(1-c1), s2p = c2*(1-c2)

Sharding: data-parallel over the batch dim across 8 NeuronCores (64 rows
per core); W1/W2/biases replicated. Everything on-chip per core.

Per-core layout strategy (partition dim first):
  - encoder computed transposed (hidden dim on partitions) so that
    s1p/s2p land as per-partition scalar columns:
        c1T [H1, BL], c2T [H2, BL], c3T [H1, BL]
  - Jac[b] = M_b @ W1 with M_bT = W2T * s1p[b] (per-partition scale,
    DVE, cast to fp16) as the matmul stationary operand and W1 (natural
    layout, fp16) as the moving operand; the s2p[b] scale is fused into
    the mandatory PSUM->SBUF copy (ACT, per-partition scale).
  - derivative tensors are stored negated ((c-1)*c = -c(1-c), one DVE
    op each); the two negations cancel in Jac.
  - encoder matmuls + transposes in fp32 for accuracy (tiny FLOP count);
    the 8.6 GFLOP/core Jacobian einsum and the recover matmul run in
    fp16 (same 1 cycle/row PE rate as bf16, 8x finer mantissa; all
    values are far inside fp16 range).
"""

from contextlib import ExitStack

import numpy as np

import concourse.bass as bass
import concourse.tile as tile
from concourse import bacc
from concourse import masks, mybir
from concourse.bass_utils import run_bass_kernel_spmd

F32 = mybir.dt.float32
F16 = mybir.dt.float16  # same PE rate as bf16, 8x finer mantissa
AF = mybir.ActivationFunctionType
ALU = mybir.AluOpType

B, D, H1, H2 = 512, 1024, 512, 128
NCORES = 8
BL = B // NCORES  # 64 batch rows per core
P = 128           # partitions
K1 = H1 // P      # 4 k-tiles over H1
KD = D // P       # 8 k-tiles over D
NB = 512          # fp32 psum bank width (free dim)
ND = D // NB      # 2 n-tiles over D
JDMA = 2          # jac batches per output DMA (1MiB each)


def _build() -> bass.Bass:
    nc = bacc.Bacc("TRN2")

    x_d = nc.dram_tensor("x", [BL, D], F32, kind="ExternalInput")
    w1_d = nc.dram_tensor("w1", [H1, D], F32, kind="ExternalInput")
    b1_d = nc.dram_tensor("b1", [H1], F32, kind="ExternalInput")
    w2_d = nc.dram_tensor("w2", [H2, H1], F32, kind="ExternalInput")
    b2_d = nc.dram_tensor("b2", [H2], F32, kind="ExternalInput")
    b3_d = nc.dram_tensor("b3", [H1], F32, kind="ExternalInput")
    br_d = nc.dram_tensor("br", [D], F32, kind="ExternalInput")

    rec_d = nc.dram_tensor("rec", [BL, D], F32, kind="ExternalOutput")
    c2_d = nc.dram_tensor("c2o", [BL, H2], F32, kind="ExternalOutput")
    jac_d = nc.dram_tensor("jac", [BL, H2, D], F32, kind="ExternalOutput")

    with tile.TileContext(nc) as tc, ExitStack() as ctx:
        const = ctx.enter_context(tc.tile_pool(name="const", bufs=1))
        work = ctx.enter_context(tc.tile_pool(name="work", bufs=3))
        lhsp = ctx.enter_context(tc.tile_pool(name="lhsp", bufs=4))
        jop = ctx.enter_context(tc.tile_pool(name="jop", bufs=3))
        pp = ctx.enter_context(tc.tile_pool(name="pp", bufs=4, space="PSUM"))
        jp = ctx.enter_context(tc.tile_pool(name="jp", bufs=2, space="PSUM"))

        # ---- PE warm-up: ~3us of cheap fp16 dummy matmuls so the PE clock
        # ramp (HAM) is fully open by the time the first input DMA lands;
        # results are discarded ----
        warm = const.tile([1, NB], F16, name="warm", tag="warm")
        nc.gpsimd.memset(warm[:], 0.0)

        ident = const.tile([P, P], F32, name="ident", tag="ident")
        masks.make_identity(nc, ident[:])

        for _ in range(4):
            wps = pp.tile([P, NB], F32, name="wps", tag="pp")
            nc.tensor.matmul(wps[:], lhsT=warm[:, :P], rhs=warm[:],
                             start=True, stop=True)

        # ---- load x first (small, one DMA) so PE can start transposing ----
        xin = work.tile([BL, D], F32, name="xin", tag="xin", bufs=1)
        nc.sync.dma_start(xin[:], x_d[:, :])

        # ---- load weights / biases ----
        w1f = []
        for k in range(K1):
            t = const.tile([P, D], F32, name=f"w1f{k}", tag=f"w1f{k}")
            nc.sync.dma_start(t[:, :D // 2], w1_d[k * P:(k + 1) * P, :D // 2])
            nc.sync.dma_start(t[:, D // 2:], w1_d[k * P:(k + 1) * P, D // 2:])
            w1f.append(t)
        w2f = const.tile([P, H1], F32, name="w2f", tag="w2f")
        nc.sync.dma_start(w2f[:], w2_d[:, :])

        b1t = const.tile([P, K1], F32, name="b1t", tag="b1t")
        nc.sync.dma_start(b1t[:], b1_d[:].rearrange("(k p) -> p k", p=P))
        b3t = const.tile([P, K1], F32, name="b3t", tag="b3t")
        nc.sync.dma_start(b3t[:], b3_d[:].rearrange("(k p) -> p k", p=P))
        b2t = const.tile([P, 1], F32, name="b2t", tag="b2t")
        nc.sync.dma_start(b2t[:], b2_d[:].rearrange("(p o) -> p o", o=1))
        brf = const.tile([1, D], F32, name="brf", tag="brf")
        nc.sync.dma_start(brf[:], br_d[:].rearrange("(o d) -> o d", o=1))
        brt = const.tile([1, D], F16, name="brt", tag="brt")
        nc.vector.tensor_copy(brt[:], brf[:])
        ones_t = const.tile([1, BL], F16, name="ones", tag="ones")
        nc.vector.memset(ones_t[:], 1.0)

        # ---- W1 cast to fp16 (moving operand of the Jacobian matmuls) ----
        w1b = []
        for k in range(K1):
            t = const.tile([P, D], F16, name=f"w1b{k}", tag=f"w1b{k}")
            nc.scalar.copy(t[:], w1f[k][:])
            w1b.append(t)

        # ---- transposes (PE identity-matmul, fp32), packed 4-8 per PSUM
        # bank with a single wide DVE drain per bank (avoids per-transpose
        # slot/semaphore churn) ----
        # x [BL, D] -> xT_all [P, KD*BL]; column block k = xT of d-tile k
        xT_all = const.tile([P, KD * BL], F32, name="xT_all", tag="xT_all")
        psx = pp.tile([P, KD * BL], F32, name="psx", tag="pp")
        for k in range(KD):
            nc.tensor.transpose(psx[:, k * BL:(k + 1) * BL],
                                xin[:, k * P:(k + 1) * P], ident[:BL, :BL])
        nc.vector.tensor_copy(xT_all[:], psx[:])
        xT = [xT_all[:, k * BL:(k + 1) * BL] for k in range(KD)]

        # W1 [H1, D] -> w1T_all [P, K1*KD*P]; column block (m*KD+k) holds
        # (W1[m-block, k-block]).T
        w1T_all = const.tile([P, K1 * KD * P], F32, name="w1T_all", tag="w1T_all")
        for m in range(K1):
            for kg in range(0, KD, 4):
                ps = pp.tile([P, 4 * P], F32, name="pst", tag="pp")
                for dk in range(4):
                    k = kg + dk
                    nc.tensor.transpose(ps[:, dk * P:(dk + 1) * P],
                                        w1f[m][:, k * P:(k + 1) * P], ident[:])
                nc.vector.tensor_copy(
                    w1T_all[:, (m * KD + kg) * P:(m * KD + kg + 4) * P], ps[:])

        def w1T(k, m):
            return w1T_all[:, (m * KD + k) * P:(m * KD + k + 1) * P]

        # W2 [H2, H1] -> w2T_all [P, K1*H2]; column block k = W2[:, k-block].T
        w2T_all = const.tile([P, K1 * H2], F32, name="w2T_all", tag="w2T_all")
        psw = pp.tile([P, K1 * H2], F32, name="psw", tag="pp")
        for k in range(K1):
            nc.tensor.transpose(psw[:, k * H2:(k + 1) * H2],
                                w2f[:, k * P:(k + 1) * P], ident[:])
        nc.vector.tensor_copy(w2T_all[:], psw[:])
        w2T = [w2T_all[:, k * H2:(k + 1) * H2] for k in range(K1)]

        # ---- encoder (fp32) ----
        # c1T[m] = sigmoid(W1 @ xT + b1)   [P, BL] per H1-tile m
        c1T = [const.tile([P, BL], F32, name=f"c1T{m}", tag=f"c1T{m}") for m in range(K1)]
        s1n = [const.tile([P, BL], F32, name=f"s1n{m}", tag=f"s1n{m}") for m in range(K1)]  # -s1p
        for m in range(K1):
            ps = pp.tile([P, BL], F32, name="pp", tag="pp")
            for k in range(KD):
                nc.tensor.matmul(
                    ps[:], lhsT=w1T(k, m), rhs=xT[k][:],
                    start=(k == 0), stop=(k == KD - 1),
                )
            nc.scalar.activation(c1T[m][:], ps[:], AF.Sigmoid, bias=b1t[:, m:m + 1])
            # -s1p = (c1 - 1) * c1
            nc.vector.scalar_tensor_tensor(
                s1n[m][:], c1T[m][:], 1.0, c1T[m][:], ALU.subtract, ALU.mult,
            )

        # c2T = sigmoid(W2 @ c1T... ) : lhsT = W2T tiles, rhs = c1T tiles
        c2Tt = const.tile([P, BL], F32, name="c2T", tag="c2T")
        s2n = const.tile([P, BL], F32, name="s2n", tag="s2n")  # -s2p
        ps = pp.tile([P, BL], F32, name="pp", tag="pp")
        for k in range(K1):
            nc.tensor.matmul(
                ps[:], lhsT=w2T[k][:], rhs=c1T[k][:],
                start=(k == 0), stop=(k == K1 - 1),
            )
        nc.scalar.activation(c2Tt[:], ps[:], AF.Sigmoid, bias=b2t[:, 0:1])
        nc.vector.scalar_tensor_tensor(
            s2n[:], c2Tt[:], 1.0, c2Tt[:], ALU.subtract, ALU.mult,
        )

        def emit_decoder():
            # c2 natural output [BL, H2]
            psc = pp.tile([BL, H2], F32, name="psc", tag="pp")
            nc.tensor.transpose(psc[:], c2Tt[:], ident[:])
            c2n = work.tile([BL, H2], F32, name="c2n", tag="c2n")
            nc.vector.tensor_copy(c2n[:], psc[:])
            nc.sync.dma_start(c2_d[:, :], c2n[:])

            # c3T[m] = sigmoid(W2.T @ c2T + b3) : lhsT = W2 natural slices
            # (fp16 output: feeds the fp16 recover matmul)
            c3T = [const.tile([P, BL], F16, name=f"c3T{m}", tag=f"c3T{m}")
                   for m in range(K1)]
            for m in range(K1):
                ps = pp.tile([P, BL], F32, name="psd", tag="pp")
                nc.tensor.matmul(
                    ps[:], lhsT=w2f[:, m * P:(m + 1) * P], rhs=c2Tt[:],
                    start=True, stop=True,
                )
                nc.scalar.activation(c3T[m][:], ps[:], AF.Sigmoid, bias=b3t[:, m:m + 1])

            # recover = c3 @ W1 + b_r  (natural [BL, D], fp16 matmul;
            # b_r added via a K=1 rank-1 matmul into the same PSUM group)
            rec_sb = work.tile([BL, D], F32, name="rec", tag="rec")
            for n in range(ND):
                ps = pp.tile([BL, NB], F32, name="psr", tag="pp")
                for k in range(K1):
                    nc.tensor.matmul(
                        ps[:], lhsT=c3T[k][:], rhs=w1b[k][:, n * NB:(n + 1) * NB],
                        start=(k == 0), stop=False,
                    )
                nc.tensor.matmul(
                    ps[:], lhsT=ones_t[:], rhs=brt[:, n * NB:(n + 1) * NB],
                    start=False, stop=True,
                )
                nc.vector.tensor_copy(rec_sb[:, n * NB:(n + 1) * NB], ps[:])
            nc.sync.dma_start(rec_d[:, :], rec_sb[:])

        # ---- Jacobian loop (fp16 matmuls) ----
        # (the decoder/recover emission is deferred into the loop so the Jac
        # output DMA stream starts as early as possible)
        # Jac[b] = (-s2p[b]) * [ ((W2T * -s1p[b]).T ) @ W1 ]
        # k-outer / n-inner: each stationary lhs tile is loaded once and
        # streams both 512-wide halves of W1.
        for b0 in range(0, BL, JDMA):
            jt = jop.tile([P, JDMA * D], F32, name="jo", tag="jo")
            for bi in range(JDMA):
                b = b0 + bi
                lhs = []
                for k in range(K1):
                    lt = lhsp.tile([P, H2], F16, name=f"lhs{k}", tag=f"lhs{k}")
                    nc.vector.tensor_scalar_mul(lt[:], w2T[k][:], s1n[k][:, b:b + 1])
                    lhs.append(lt)
                pss = [jp.tile([P, NB], F32, name=f"jp{n}", tag=f"jp{n}")
                       for n in range(ND)]
                for k in range(K1):
                    for n in range(ND):
                        nc.tensor.matmul(
                            pss[n][:], lhsT=lhs[k][:],
                            rhs=w1b[k][:, n * NB:(n + 1) * NB],
                            start=(k == 0), stop=(k == K1 - 1),
                        )
                last = b >= BL - 2
                for n in range(ND):
                    dst = jt[:, bi * D + n * NB: bi * D + (n + 1) * NB]
                    if last and n == 1:
                        # split the final batch's drain across ACT+DVE so the
                        # kernel tail is shorter
                        nc.vector.tensor_scalar_mul(dst, pss[n][:], s2n[:, b:b + 1])
                    else:
                        nc.scalar.mul(dst, pss[n][:], s2n[:, b:b + 1])
            if b0 >= BL - 2 * JDMA:
                for bi in range(JDMA):
                    nc.sync.dma_start(
                        jac_d[b0 + bi],
                        jt[:, bi * D:(bi + 1) * D],
                    )
            else:
                nc.sync.dma_start(
                    jac_d[b0:b0 + JDMA].rearrange("b h d -> h b d"),
                    jt[:].rearrange("p (b d) -> p b d", b=JDMA),
                )
            if b0 == 0:
                emit_decoder()

    nc.compile()
    return nc


_CACHE: dict = {}


def _get_nc() -> bass.Bass:
    if "nc" not in _CACHE:
        _CACHE["nc"] = _build()
    return _CACHE["nc"]


def _in_maps(x, W1, b1, W2, b2, b3, b_r):
    def f(a):
        return np.ascontiguousarray(np.asarray(a, dtype=np.float32))

    x, W1, b1, W2, b2, b3, b_r = map(f, (x, W1, b1, W2, b2, b3, b_r))
    maps = []
    for c in range(NCORES):
        maps.append({
            "x": x[c * BL:(c + 1) * BL],
            "w1": W1, "b1": b1, "w2": W2, "b2": b2, "b3": b3, "br": b_r,
        })
    return maps


def _gather(results):
    recover = np.concatenate([results[c]["rec"] for c in range(NCORES)], axis=0)
    c2 = np.concatenate([results[c]["c2o"] for c in range(NCORES)], axis=0)
    jac = np.concatenate([results[c]["jac"] for c in range(NCORES)], axis=0)
    return recover, c2, jac


def kernel(x, W1, b1, W2, b2, b3, b_r):
    nc = _get_nc()
    maps = _in_maps(x, W1, b1, W2, b2, b3, b_r)
    res = run_bass_kernel_spmd(nc, maps, list(range(NCORES)))
    return _gather(res.results)


def kernel_profiled(x, W1, b1, W2, b2, b3, b_r, **kw):
    """Same as kernel() but with NTFF tracing; returns (outputs, results)."""
    nc = _get_nc()
    maps = _in_maps(x, W1, b1, W2, b2, b3, b_r)
    res = run_bass_kernel_spmd(nc, maps, list(range(NCORES)), trace=True, **kw)
    return _gather(res.results), res


def _make_sharded(nc):
    """Build the same shard_map'ed PJRT callable bass2jax uses, without
    output donation, so it can be re-invoked for timing."""
    import jax
    import numpy as jnp_np
    from jax.sharding import Mesh, PartitionSpec
    from jax.experimental.shard_map import shard_map
    from concourse import bass2jax, mybir as mb

    bass2jax.install_neuronx_cc_hook()
    in_names, out_names, out_avals = [], [], []
    partition_name = nc.partition_id_tensor.name if nc.partition_id_tensor else None
    for alloc in nc.m.functions[0].allocations:
        if not isinstance(alloc, mb.MemoryLocationSet):
            continue
        name = alloc.memorylocations[0].name
        if alloc.kind == "ExternalInput":
            if name != partition_name:
                in_names.append(name)
        elif alloc.kind == "ExternalOutput":
            out_names.append(name)
            out_avals.append(jax.core.ShapedArray(
                tuple(alloc.tensor_shape), mb.dt.np(alloc.dtype)))
    n_params = len(in_names)
    all_in_names = list(in_names) + list(out_names)
    if partition_name is not None:
        all_in_names.append(partition_name)

    def _body(*args):
        operands = list(args)
        if partition_name is not None:
            operands.append(bass2jax.partition_id_tensor())
        return tuple(bass2jax._bass_exec_p.bind(
            *operands,
            out_avals=tuple(out_avals),
            in_names=tuple(all_in_names),
            out_names=tuple(out_names),
            lowering_input_output_aliases=(),
            sim_require_finite=True,
            sim_require_nnan=True,
            nc=nc,
        ))

    devices = jax.devices()[:NCORES]
    mesh = Mesh(jnp_np.asarray(devices), ("core",))
    n_outs = len(out_names)
    in_specs = (PartitionSpec("core"),) * (n_params + n_outs)
    out_specs = (PartitionSpec("core"),) * n_outs
    fn = jax.jit(shard_map(_body, mesh=mesh, in_specs=in_specs,
                           out_specs=out_specs, check_rep=False),
                 keep_unused=True)
    return fn, in_names, out_names, out_avals


def measure_chain_ns(x, W1, b1, W2, b2, b3, b_r, chain=8, iters=8):
    """Estimate the marginal on-device execution time of one kernel run by
    timing a jitted program that chains `chain` data-dependent kernel
    executions, vs one with a single execution. The axon-tunnel dispatch
    overhead (~100ms) cancels in the difference."""
    import time as _time

    import jax
    import jax.numpy as jnp
    import numpy as jnp_np
    from jax.sharding import Mesh, PartitionSpec
    from jax.experimental.shard_map import shard_map
    from concourse import bass2jax, mybir as mb

    nc = _get_nc()
    maps = _in_maps(x, W1, b1, W2, b2, b3, b_r)
    bass2jax.install_neuronx_cc_hook()

    in_names, out_names, out_avals = [], [], []
    partition_name = nc.partition_id_tensor.name if nc.partition_id_tensor else None
    for alloc in nc.m.functions[0].allocations:
        if not isinstance(alloc, mb.MemoryLocationSet):
            continue
        name = alloc.memorylocations[0].name
        if alloc.kind == "ExternalInput":
            if name != partition_name:
                in_names.append(name)
        elif alloc.kind == "ExternalOutput":
            out_names.append(name)
            out_avals.append(jax.core.ShapedArray(
                tuple(alloc.tensor_shape), mb.dt.np(alloc.dtype)))
    all_in_names = list(in_names) + list(out_names)
    if partition_name is not None:
        all_in_names.append(partition_name)
    n_params = len(in_names)
    i_x = in_names.index("x")
    i_rec = out_names.index("rec")

    def _one(ins, zeros):
        operands = list(ins) + list(zeros)
        if partition_name is not None:
            operands.append(bass2jax.partition_id_tensor())
        return bass2jax._bass_exec_p.bind(
            *operands,
            out_avals=tuple(out_avals),
            in_names=tuple(all_in_names),
            out_names=tuple(out_names),
            lowering_input_output_aliases=(),
            sim_require_finite=True,
            sim_require_nnan=True,
            nc=nc,
        )

    def _make(n):
        def _body(*args):
            ins = list(args[:n_params])
            zeros = list(args[n_params:])
            outs = _one(ins, zeros)
            for _ in range(n - 1):
                # pure data dependency: feed rec back as x (same shape/dtype)
                ins = list(ins)
                ins[i_x] = outs[i_rec]
                outs = _one(ins, zeros)
            return tuple(outs)
        devices = jax.devices()[:NCORES]
        mesh = Mesh(jnp_np.asarray(devices), ("core",))
        n_outs = len(out_names)
        return jax.jit(shard_map(
            _body, mesh=mesh,
            in_specs=(PartitionSpec("core"),) * (n_params + n_outs),
            out_specs=(PartitionSpec("core"),) * n_outs, check_rep=False),
            keep_unused=True)

    concat_in = [
        jnp_np.concatenate([jnp_np.asarray(maps[c][n]) for c in range(NCORES)], axis=0)
        for n in in_names
    ] + [
        jnp_np.zeros((NCORES * a.shape[0], *a.shape[1:]), a.dtype) for a in out_avals
    ]
    dev_in = [jax.device_put(a) for a in concat_in]
    jax.block_until_ready(dev_in)

    def time_fn(fn):
        jax.block_until_ready(fn(*dev_in))  # compile+warm
        ts = []
        for _ in range(iters):
            t0 = _time.perf_counter()
            jax.block_until_ready(fn(*dev_in))
            ts.append((_time.perf_counter() - t0) * 1e9)
        ts.sort()
        return ts

    t1 = time_fn(_make(1))
    tn = time_fn(_make(chain))
    per = (tn[0] - t1[0]) / (chain - 1)
    per_med = (tn[len(tn) // 2] - t1[len(t1) // 2]) / (chain - 1)
    return per, per_med, t1, tn


def measure_exec_ns(x, W1, b1, W2, b2, b3, b_r, iters=20):
    """Warm wall-clock of the sharded PJRT executable with device-resident
    inputs. Returns (min_ns, median_ns, all_ns)."""
    import time as _time

    import jax
    import numpy as jnp_np

    nc = _get_nc()
    maps = _in_maps(x, W1, b1, W2, b2, b3, b_r)
    fn, in_names, out_names, out_avals = _make_sharded(nc)
    concat_in = [
        jnp_np.concatenate([jnp_np.asarray(maps[c][n]) for c in range(NCORES)], axis=0)
        for n in in_names
    ]
    concat_zeros = [
        jnp_np.zeros((NCORES * a.shape[0], *a.shape[1:]), a.dtype) for a in out_avals
    ]
    dev_in = [jax.device_put(a) for a in concat_in + concat_zeros]
    jax.block_until_ready(dev_in)

    outs = fn(*dev_in)   # compile + warm
    jax.block_until_ready(outs)

    times = []
    for _ in range(iters):
        t0 = _time.perf_counter()
        outs = fn(*dev_in)
        jax.block_until_ready(outs)
        times.append((_time.perf_counter() - t0) * 1e9)
    times.sort()
    return times[0], times[len(times) // 2], times, outs, out_names
